# revision 6
# baseline (speedup 1.0000x reference)
"""BitNet-style binary linear: y = x @ w_q.T + bias, w_q = clip(round(w/g))*g.

Strategy (8 NeuronCores, tensor-parallel on out_features):
  - Host: g = max(mean|w|, 1e-5); s = clip(rint(w/g), -1, 1). s is ternary so
    it is EXACT in fp8e4. x is scaled by SC=32 and split along K: the first
    N16=16 k-chunks are fp16 (exact to ~2e-4), the remaining 16 chunks are
    fp8e4m3 and computed pairwise with perf_mode=DoubleRow at 2x PE rate
    (measured 228 ns per K=256/N=512 slab vs 438 ns at the 16-bit rate).
    End-to-end l2 relative error 1.88e-2 (gate 2e-2), deterministic.
  - Shard s rows (out_features) 8-ways; replicate x. Each core computes
    out[8192, 2048] = xs @ s_shard.T with all of s_shard.T resident in SBUF
    (8 MB fp8) and x streamed in r-blocks (fp16 half + fp8 half).
  - PSUM eviction fuses the gamma/SC rescale and the bias add in one DVE
    scalar_tensor_tensor: out = (psum * gs) + bias_bc.
  - Host packs every tensor into the exact SBUF tile layout
    [128 partitions, k-chunk, cols] so every DMA is fully contiguous.
  - Pipeline priming: weights arrive in 4 f-quarters; the first 512 rows are
    processed one f-quarter at a time so the in-order PE always has work
    while later quarters stream in.
"""

import numpy as np

B, S, D_IN, D_OUT = 4, 2048, 4096, 16384
N_CORES = 8
R = B * S                 # 8192 rows of x
F = D_OUT // N_CORES      # 2048 features per core
KC = D_IN // 128          # 32 k-chunks
N16 = 16                  # k-chunks computed in fp16 (chunks 0..N16-1)
KC8 = KC - N16            # k-chunks computed in fp8 DoubleRow pairs
SC = 32.0                 # x pre-scale (power of 2; undone at eviction)
RB = 512                  # steady-state r-block
FT = 512                  # f-tile (one PSUM bank)
NF = F // FT              # 4 f-tiles == wt quarters
NB = (R - 512) // RB      # 15 steady blocks (rows 512..8192)

_CACHE = {}


def _patch_light_exit():
    """Drop the second all-engine barrier in TileContext's exit: sem clears
    run in each engine's own stream and NRT waits for stream completion
    before any re-execution, so the trailing butterfly only adds ~3us."""
    import concourse.tile as tile
    from concourse.vector_clock import ScopedClock

    if getattr(tile.TileContext, "_light_exit", False):
        return

    def _drain_and_barrier(self, tick_clock, wait_clock):
        nc = self.nc
        drain_inst = nc.sync.drain()
        wait_clock.add_sem_waits(
            drain_inst.ins, ScopedClock({None: tick_clock.global_clock})
        )
        nc.all_engine_barrier()
        popped = nc._tile_sem_poison_stack.pop()
        assert popped is self._sem_poison
        nc.clear_and_free_semaphores(list(self.sems.allocated().values()))

    tile.TileContext._drain_and_barrier = _drain_and_barrier
    tile.TileContext._light_exit = True


def _build_nc():
    import concourse.mybir as mybir
    import concourse.tile as tile
    from concourse import bacc

    _patch_light_exit()
    fp16 = mybir.dt.float16
    fp8 = mybir.dt.float8e4
    f32 = mybir.dt.float32
    DR = mybir.MatmulPerfMode.DoubleRow
    MULT, ADD = mybir.AluOpType.mult, mybir.AluOpType.add

    nc = bacc.Bacc("TRN2", target_bir_lowering=False, debug=False,
                   num_devices=N_CORES)
    xh16_0 = nc.declare_dram_parameter("xh16_0", [128, N16, 128], fp16, isOutput=False)
    xh16_1 = nc.declare_dram_parameter("xh16_1", [128, N16, 384], fp16, isOutput=False)
    xp16 = nc.declare_dram_parameter("xp16", [NB, 128, N16, RB], fp16, isOutput=False)
    xh8_0 = nc.declare_dram_parameter("xh8_0", [128, KC8, 128], fp8, isOutput=False)
    xh8_1 = nc.declare_dram_parameter("xh8_1", [128, KC8, 384], fp8, isOutput=False)
    xp8 = nc.declare_dram_parameter("xp8", [NB, 128, KC8, RB], fp8, isOutput=False)
    wq = nc.declare_dram_parameter("wq", [NF, 128, KC, FT], fp8, isOutput=False)
    bias = nc.declare_dram_parameter("bias", [1, F], fp16, isOutput=False)
    gs = nc.declare_dram_parameter("gs", [128, 1], f32, isOutput=False)
    out = nc.declare_dram_parameter("out", [R, F], f32, isOutput=True)

    with tile.TileContext(nc) as tc:
        with (
            tc.tile_pool(name="wpool", bufs=1) as wpool,
            tc.tile_pool(name="cpool", bufs=1) as cpool,
            tc.tile_pool(name="xpool", bufs=2) as xpool,
            tc.tile_pool(name="opool", bufs=8) as opool,
            tc.tile_pool(name="pspool", bufs=4, space="PSUM") as pspool,
        ):
            # Two HWDGE queues: x rows ride the SP (sync) queue; weights,
            # bias and all output tiles ride the Activation (scalar) queue.
            # Each stream is issued just-in-time order for the prime phase.
            bias_sb = cpool.tile([1, F], fp16, tag="bias")
            nc.scalar.dma_start(bias_sb[:], bias[:, :])
            gs_sb = cpool.tile([128, 1], f32, tag="gs")
            nc.scalar.dma_start(gs_sb[:], gs[:, :])

            xh16_0t = cpool.tile([128, N16, 128], fp16, tag="xh16_0")
            nc.sync.dma_start(xh16_0t[:], xh16_0[:, :, :])
            xh8_0t = cpool.tile([128, KC8, 128], fp8, tag="xh8_0")
            nc.sync.dma_start(xh8_0t[:], xh8_0[:, :, :])
            # 384-row head block: split per r-tile so tile N+1's matmuls
            # only wait on their own 128 rows
            xh16_1t = cpool.tile([128, N16, 384], fp16, tag="xh16_1")
            xh8_1t = cpool.tile([128, KC8, 384], fp8, tag="xh8_1")
            for j in range(3):
                nc.sync.dma_start(xh16_1t[:, :, j * 128:(j + 1) * 128],
                                  xh16_1[:, :, j * 128:(j + 1) * 128])
                nc.sync.dma_start(xh8_1t[:, :, j * 128:(j + 1) * 128],
                                  xh8_1[:, :, j * 128:(j + 1) * 128])
            # weight quarters, each split into the fp16-chunk half (needed
            # first) and the fp8-chunk half
            wt_sb = []
            for q in range(NF):
                t = wpool.tile([128, KC, FT], fp8, tag=f"wq{q}")
                nc.scalar.dma_start(t[:, 0:N16, :], wq[q, :, 0:N16, :])
                nc.scalar.dma_start(t[:, N16:KC, :], wq[q, :, N16:KC, :])
                wt_sb.append(t)

            # broadcast bias across partitions: ones[1,128].T @ bias[1,512].
            # Repeat the matmuls 4x: ~3.5us of back-to-back PE work while the
            # first DMAs stream in, pushing HAM through the K=4/8 throttle
            # window so the prime tiles run at full rate.
            ones_sb = cpool.tile([1, 128], fp16, tag="ones")
            nc.vector.memset(ones_sb[:], 1.0)
            bias_bc = cpool.tile([128, F], f32, tag="bias_bc")
            bps = [pspool.tile([128, FT], f32, name=f"bp{f}", bufs=1)
                   for f in range(NF)]
            for r in range(4):
                for f in range(NF):
                    nc.tensor.matmul(bps[f][:], ones_sb[:],
                                     bias_sb[:, f * FT:(f + 1) * FT],
                                     start=True, stop=True)
            for f in range(NF):
                nc.vector.tensor_copy(bias_bc[:, f * FT:(f + 1) * FT],
                                      bps[f][:])

            def do_tile(xt16, xt8, rt, r0, f, split_evict=False):
                wt = wt_sb[f]
                ps = pspool.tile([128, FT], f32)
                c0 = rt * 128
                for c in range(N16):
                    nc.tensor.matmul(
                        ps[:],
                        xt16[:, c, c0:c0 + 128],
                        wt[:, c, :],
                        start=(c == 0), stop=False,
                    )
                for j in range(KC8 // 2):
                    nc.tensor.matmul(
                        ps[:],
                        xt8[:, 2 * j:2 * j + 2, c0:c0 + 128],
                        wt[:, N16 + 2 * j:N16 + 2 * j + 2, :],
                        start=False, stop=(j == KC8 // 2 - 1),
                        perf_mode=DR,
                    )
                ob = opool.tile([128, FT], f32)
                f0 = f * FT
                # split_evict halves the DVE->DMA latency chain; used for the
                # final tiles so the kernel's drain tail is shorter
                for lo, hi in ([(0, 256), (256, 512)] if split_evict
                               else [(0, 512)]):
                    nc.vector.scalar_tensor_tensor(
                        ob[:, lo:hi], ps[:, lo:hi], gs_sb[:, 0:1],
                        bias_bc[:, f0 + lo:f0 + hi],
                        op0=MULT, op1=ADD,
                    )
                    nc.scalar.dma_start(
                        out[r0:r0 + 128, f0 + lo:f0 + hi], ob[:, lo:hi]
                    )

            # prime: rows 0..512, one f-quarter at a time (PE is in-order;
            # quarter f+1 streams in while quarter f computes)
            for f in range(NF):
                do_tile(xh16_0t, xh8_0t, 0, 0, f)
                for rt in range(3):
                    do_tile(xh16_1t, xh8_1t, rt, 128 + rt * 128, f)

            # steady state
            for b in range(NB):
                xt16 = xpool.tile([128, N16, RB], fp16)
                nc.sync.dma_start(xt16[:], xp16[b, :, :, :])
                xt8 = xpool.tile([128, KC8, RB], fp8)
                nc.sync.dma_start(xt8[:], xp8[b, :, :, :])
                last = b == NB - 1
                for rt in range(RB // 128):
                    for f in range(NF):
                        do_tile(xt16, xt8, rt, 512 + b * RB + rt * 128, f,
                                split_evict=(last and rt == 3))
    nc.compile()
    return nc


def _pack(a):
    """[rows, k] -> [128, k//128, rows] in SBUF layout (partition = k%128)."""
    rows = a.shape[0]
    kc = a.shape[1] // 128
    return np.ascontiguousarray(a.T.reshape(kc, 128, rows).transpose(1, 0, 2))


def _prepare_in_maps(x, weight, bias):
    import ml_dtypes

    x = np.asarray(x)
    weight = np.asarray(weight)
    bias = np.asarray(bias)

    gamma = np.float32(max(np.mean(np.abs(weight), dtype=np.float64), 1e-5))
    s = np.clip(np.rint(weight.astype(np.float32) / gamma), -1.0, 1.0)

    xs = x.reshape(R, D_IN) * np.float32(SC)
    k16 = N16 * 128
    xs16 = xs[:, :k16].astype(np.float16)
    xs8 = xs[:, k16:].astype(ml_dtypes.float8_e4m3)

    xh16_0 = _pack(xs16[0:128])
    xh16_1 = _pack(xs16[128:512])
    xp16 = np.stack([_pack(xs16[512 + b * RB:512 + (b + 1) * RB])
                     for b in range(NB)])
    xh8_0 = _pack(xs8[0:128])
    xh8_1 = _pack(xs8[128:512])
    xp8 = np.stack([_pack(xs8[512 + b * RB:512 + (b + 1) * RB])
                    for b in range(NB)])

    gs = np.full((128, 1), gamma / np.float32(SC), dtype=np.float32)
    b16 = bias.astype(np.float16)
    in_maps = []
    for c in range(N_CORES):
        sh = s[c * F:(c + 1) * F].astype(ml_dtypes.float8_e4m3)  # [F, D_IN]
        wqq = np.stack([_pack(sh[q * FT:(q + 1) * FT, :]) for q in range(NF)])
        in_maps.append({
            "xh16_0": xh16_0, "xh16_1": xh16_1, "xp16": xp16,
            "xh8_0": xh8_0, "xh8_1": xh8_1, "xp8": xp8,
            "wq": wqq, "gs": gs,
            "bias": np.ascontiguousarray(b16[c * F:(c + 1) * F]).reshape(1, F),
        })
    return in_maps


def _assemble(results):
    out = np.concatenate([results[c]["out"] for c in range(N_CORES)], axis=1)
    return out.reshape(B, S, D_OUT)


def kernel(x, weight, bias):
    import os
    import time
    os.environ.setdefault("BASS_NEVER_TRACE", "1")
    from concourse.bass_utils import run_bass_kernel_spmd

    in_maps = _prepare_in_maps(x, weight, bias)
    if "nc" not in _CACHE:
        _CACHE["nc"] = _build_nc()
    last_err = None
    for attempt in range(3):
        try:
            res = run_bass_kernel_spmd(
                _CACHE["nc"], in_maps, core_ids=list(range(N_CORES)))
            return _assemble(res.results)
        except Exception as e:  # transient device errors (e.g. prior process
            last_err = e        # still tearing down) clear after ~30s
            time.sleep(30 * (attempt + 1))
    raise last_err


if __name__ == "__main__":
    import jax
    jax.config.update("jax_platforms", "cpu")
    import reference

    inputs = reference.setup_inputs()
    expected = np.asarray(reference.reference(**inputs))
    actual = kernel(**{k: np.asarray(v) for k, v in inputs.items()})
    err = actual.astype(np.float64) - expected.astype(np.float64)
    l2 = np.sqrt((err ** 2).mean()) / np.sqrt(
        (expected.astype(np.float64) ** 2).mean())
    print(f"Relative error: {l2:.6e}")


# revision 8
# speedup vs baseline: 1.0190x; 1.0190x over previous
"""BitNet-style binary linear: y = x @ w_q.T + bias, w_q = clip(round(w/g))*g.

Strategy (8 NeuronCores, tensor-parallel on out_features):
  - Host: g = max(mean|w|, 1e-5); s = clip(rint(w/g), -1, 1). s is ternary so
    it is EXACT in fp8e4. x is scaled by SC=32 and split along K: the first
    N16=16 k-chunks are fp16 (exact to ~2e-4), the remaining 16 chunks are
    fp8e4m3 and computed pairwise with perf_mode=DoubleRow at 2x PE rate
    (measured 228 ns per K=256/N=512 slab vs 438 ns at the 16-bit rate).
    End-to-end l2 relative error 1.88e-2 (gate 2e-2), deterministic.
  - Shard s rows (out_features) 8-ways; replicate x. Each core computes
    out[8192, 2048] = xs @ s_shard.T with all of s_shard.T resident in SBUF
    (8 MB fp8) and x streamed in r-blocks (fp16 half + fp8 half).
  - PSUM eviction fuses the gamma/SC rescale and the bias add in one DVE
    scalar_tensor_tensor: out = (psum * gs) + bias_bc.
  - Host packs every tensor into the exact SBUF tile layout
    [128 partitions, k-chunk, cols] so every DMA is fully contiguous.
  - Pipeline priming: weights arrive in 4 f-quarters; the first 512 rows are
    processed one f-quarter at a time so the in-order PE always has work
    while later quarters stream in.
"""

import numpy as np

B, S, D_IN, D_OUT = 4, 2048, 4096, 16384
N_CORES = 8
R = B * S                 # 8192 rows of x
F = D_OUT // N_CORES      # 2048 features per core
KC = D_IN // 128          # 32 k-chunks
N16 = 16                  # k-chunks computed in fp16 (chunks 0..N16-1)
KC8 = KC - N16            # k-chunks computed in fp8 DoubleRow pairs
SC = 32.0                 # x pre-scale (power of 2; undone at eviction)
RB = 512                  # steady-state r-block
FT = 512                  # f-tile (one PSUM bank)
NF = F // FT              # 4 f-tiles == wt quarters
NB = (R - 512) // RB      # 15 steady blocks (rows 512..8192)

_CACHE = {}


def _patch_light_exit():
    """Drop the second all-engine barrier in TileContext's exit: sem clears
    run in each engine's own stream and NRT waits for stream completion
    before any re-execution, so the trailing butterfly only adds ~3us."""
    import concourse.tile as tile
    from concourse.vector_clock import ScopedClock

    if getattr(tile.TileContext, "_light_exit", False):
        return

    def _drain_and_barrier(self, tick_clock, wait_clock):
        nc = self.nc
        drain_inst = nc.sync.drain()
        wait_clock.add_sem_waits(
            drain_inst.ins, ScopedClock({None: tick_clock.global_clock})
        )
        nc.all_engine_barrier()
        popped = nc._tile_sem_poison_stack.pop()
        assert popped is self._sem_poison
        nc.clear_and_free_semaphores(list(self.sems.allocated().values()))

    tile.TileContext._drain_and_barrier = _drain_and_barrier
    tile.TileContext._light_exit = True


def _build_nc():
    import concourse.mybir as mybir
    import concourse.tile as tile
    from concourse import bacc

    _patch_light_exit()
    fp16 = mybir.dt.float16
    fp8 = mybir.dt.float8e4
    f32 = mybir.dt.float32
    DR = mybir.MatmulPerfMode.DoubleRow
    MULT, ADD = mybir.AluOpType.mult, mybir.AluOpType.add

    nc = bacc.Bacc("TRN2", target_bir_lowering=False, debug=False,
                   num_devices=N_CORES)
    xh16_0 = nc.declare_dram_parameter("xh16_0", [128, N16, 128], fp16, isOutput=False)
    xh16_1 = nc.declare_dram_parameter("xh16_1", [128, N16, 384], fp16, isOutput=False)
    xp16 = nc.declare_dram_parameter("xp16", [NB, 128, N16, RB], fp16, isOutput=False)
    xh8_0 = nc.declare_dram_parameter("xh8_0", [128, KC8, 128], fp8, isOutput=False)
    xh8_1 = nc.declare_dram_parameter("xh8_1", [128, KC8, 384], fp8, isOutput=False)
    xp8 = nc.declare_dram_parameter("xp8", [NB, 128, KC8, RB], fp8, isOutput=False)
    wq = nc.declare_dram_parameter("wq", [NF, 128, KC, FT], fp8, isOutput=False)
    bias = nc.declare_dram_parameter("bias", [1, F], fp16, isOutput=False)
    gs = nc.declare_dram_parameter("gs", [128, 1], f32, isOutput=False)
    out = nc.declare_dram_parameter("out", [R, F], f32, isOutput=True)

    with tile.TileContext(nc) as tc:
        with (
            tc.tile_pool(name="wpool", bufs=1) as wpool,
            tc.tile_pool(name="cpool", bufs=1) as cpool,
            tc.tile_pool(name="xpool", bufs=2) as xpool,
            tc.tile_pool(name="opool", bufs=8) as opool,
            tc.tile_pool(name="pspool", bufs=4, space="PSUM") as pspool,
        ):
            # Two HWDGE queues: x rows ride the SP (sync) queue; weights,
            # bias and all output tiles ride the Activation (scalar) queue.
            # Each stream is issued just-in-time order for the prime phase.
            # All inputs share the SP (sync) HWDGE queue so issue order IS
            # arrival priority; the order below is just-in-time for the
            # prime phase. Output tiles ride the Activation queue instead.
            bias_sb = cpool.tile([1, F], fp16, tag="bias")
            nc.sync.dma_start(bias_sb[:], bias[:, :])
            gs_sb = cpool.tile([128, 1], f32, tag="gs")
            nc.sync.dma_start(gs_sb[:], gs[:, :])

            wt_sb = [wpool.tile([128, KC, FT], fp8, name=f"wqt{q}")
                     for q in range(NF)]
            xh16_0t = cpool.tile([128, N16, 128], fp16, tag="xh16_0")
            nc.sync.dma_start(xh16_0t[:], xh16_0[:, :, :])
            # first weight quarter in halves: the fp16-chunk half unblocks
            # the first 16 matmuls after only 1 MB
            nc.sync.dma_start(wt_sb[0][:, 0:N16, :], wq[0, :, 0:N16, :])
            xh8_0t = cpool.tile([128, KC8, 128], fp8, tag="xh8_0")
            nc.sync.dma_start(xh8_0t[:], xh8_0[:, :, :])
            nc.sync.dma_start(wt_sb[0][:, N16:KC, :], wq[0, :, N16:KC, :])
            # 384-row head block: split per r-tile so tile N+1's matmuls
            # only wait on their own 128 rows
            xh16_1t = cpool.tile([128, N16, 384], fp16, tag="xh16_1")
            xh8_1t = cpool.tile([128, KC8, 384], fp8, tag="xh8_1")
            for j in range(3):
                nc.sync.dma_start(xh16_1t[:, :, j * 128:(j + 1) * 128],
                                  xh16_1[:, :, j * 128:(j + 1) * 128])
                nc.sync.dma_start(xh8_1t[:, :, j * 128:(j + 1) * 128],
                                  xh8_1[:, :, j * 128:(j + 1) * 128])
            for q in range(1, NF):
                nc.sync.dma_start(wt_sb[q][:], wq[q, :, :, :])

            # broadcast bias across partitions: ones[1,128].T @ bias[1,512].
            # Repeat the matmuls 4x: ~3.5us of back-to-back PE work while the
            # first DMAs stream in, pushing HAM through the K=4/8 throttle
            # window so the prime tiles run at full rate.
            ones_sb = cpool.tile([1, 128], fp16, tag="ones")
            nc.vector.memset(ones_sb[:], 1.0)
            bias_bc = cpool.tile([128, F], f32, tag="bias_bc")
            bps = [pspool.tile([128, FT], f32, name=f"bp{f}", bufs=1)
                   for f in range(NF)]
            for r in range(4):
                for f in range(NF):
                    nc.tensor.matmul(bps[f][:], ones_sb[:],
                                     bias_sb[:, f * FT:(f + 1) * FT],
                                     start=True, stop=True)
            for f in range(NF):
                nc.vector.tensor_copy(bias_bc[:, f * FT:(f + 1) * FT],
                                      bps[f][:])

            tile_no = [0]

            def do_tile(xt16, xt8, rt, r0, f, split_evict=False):
                wt = wt_sb[f]
                ps = pspool.tile([128, FT], f32)
                c0 = rt * 128
                # alternate fp16-first / DR-first per tile so consecutive
                # tiles meet in the same PE mode (half the mode switches)
                dr_first = tile_no[0] % 2 == 1
                tile_no[0] += 1

                def fp16_mms(start):
                    for c in range(N16):
                        nc.tensor.matmul(
                            ps[:],
                            xt16[:, c, c0:c0 + 128],
                            wt[:, c, :],
                            start=(start and c == 0),
                            stop=(not start and c == N16 - 1),
                        )

                def dr_mms(start):
                    for j in range(KC8 // 2):
                        nc.tensor.matmul(
                            ps[:],
                            xt8[:, 2 * j:2 * j + 2, c0:c0 + 128],
                            wt[:, N16 + 2 * j:N16 + 2 * j + 2, :],
                            start=(start and j == 0),
                            stop=(not start and j == KC8 // 2 - 1),
                            perf_mode=DR,
                        )

                if dr_first:
                    dr_mms(True)
                    fp16_mms(False)
                else:
                    fp16_mms(True)
                    dr_mms(False)
                ob = opool.tile([128, FT], f32)
                f0 = f * FT
                # split_evict halves the DVE->DMA latency chain; used for the
                # final tiles so the kernel's drain tail is shorter
                for lo, hi in ([(0, 256), (256, 512)] if split_evict
                               else [(0, 512)]):
                    nc.vector.scalar_tensor_tensor(
                        ob[:, lo:hi], ps[:, lo:hi], gs_sb[:, 0:1],
                        bias_bc[:, f0 + lo:f0 + hi],
                        op0=MULT, op1=ADD,
                    )
                    nc.scalar.dma_start(
                        out[r0:r0 + 128, f0 + lo:f0 + hi], ob[:, lo:hi]
                    )

            # prime: rows 0..512, one f-quarter at a time (PE is in-order;
            # quarter f+1 streams in while quarter f computes)
            for f in range(NF):
                do_tile(xh16_0t, xh8_0t, 0, 0, f)
                for rt in range(3):
                    do_tile(xh16_1t, xh8_1t, rt, 128 + rt * 128, f)

            # steady state
            for b in range(NB):
                xt16 = xpool.tile([128, N16, RB], fp16)
                nc.sync.dma_start(xt16[:], xp16[b, :, :, :])
                xt8 = xpool.tile([128, KC8, RB], fp8)
                nc.sync.dma_start(xt8[:], xp8[b, :, :, :])
                last = b == NB - 1
                for rt in range(RB // 128):
                    for f in range(NF):
                        do_tile(xt16, xt8, rt, 512 + b * RB + rt * 128, f,
                                split_evict=(last and rt == 3))
    nc.compile()
    return nc


def _pack(a):
    """[rows, k] -> [128, k//128, rows] in SBUF layout (partition = k%128)."""
    rows = a.shape[0]
    kc = a.shape[1] // 128
    return np.ascontiguousarray(a.T.reshape(kc, 128, rows).transpose(1, 0, 2))


def _prepare_in_maps(x, weight, bias):
    import ml_dtypes

    x = np.asarray(x)
    weight = np.asarray(weight)
    bias = np.asarray(bias)

    gamma = np.float32(max(np.mean(np.abs(weight), dtype=np.float64), 1e-5))
    s = np.clip(np.rint(weight.astype(np.float32) / gamma), -1.0, 1.0)

    xs = x.reshape(R, D_IN) * np.float32(SC)
    k16 = N16 * 128
    xs16 = xs[:, :k16].astype(np.float16)
    xs8 = xs[:, k16:].astype(ml_dtypes.float8_e4m3)

    xh16_0 = _pack(xs16[0:128])
    xh16_1 = _pack(xs16[128:512])
    xp16 = np.stack([_pack(xs16[512 + b * RB:512 + (b + 1) * RB])
                     for b in range(NB)])
    xh8_0 = _pack(xs8[0:128])
    xh8_1 = _pack(xs8[128:512])
    xp8 = np.stack([_pack(xs8[512 + b * RB:512 + (b + 1) * RB])
                    for b in range(NB)])

    gs = np.full((128, 1), gamma / np.float32(SC), dtype=np.float32)
    b16 = bias.astype(np.float16)
    in_maps = []
    for c in range(N_CORES):
        sh = s[c * F:(c + 1) * F].astype(ml_dtypes.float8_e4m3)  # [F, D_IN]
        wqq = np.stack([_pack(sh[q * FT:(q + 1) * FT, :]) for q in range(NF)])
        in_maps.append({
            "xh16_0": xh16_0, "xh16_1": xh16_1, "xp16": xp16,
            "xh8_0": xh8_0, "xh8_1": xh8_1, "xp8": xp8,
            "wq": wqq, "gs": gs,
            "bias": np.ascontiguousarray(b16[c * F:(c + 1) * F]).reshape(1, F),
        })
    return in_maps


def _assemble(results):
    out = np.concatenate([results[c]["out"] for c in range(N_CORES)], axis=1)
    return out.reshape(B, S, D_OUT)


def kernel(x, weight, bias):
    import os
    import time
    os.environ.setdefault("BASS_NEVER_TRACE", "1")
    from concourse.bass_utils import run_bass_kernel_spmd

    in_maps = _prepare_in_maps(x, weight, bias)
    if "nc" not in _CACHE:
        _CACHE["nc"] = _build_nc()
    last_err = None
    for attempt in range(3):
        try:
            res = run_bass_kernel_spmd(
                _CACHE["nc"], in_maps, core_ids=list(range(N_CORES)))
            return _assemble(res.results)
        except Exception as e:  # transient device errors (e.g. prior process
            last_err = e        # still tearing down) clear after ~30s
            time.sleep(30 * (attempt + 1))
    raise last_err


if __name__ == "__main__":
    import jax
    jax.config.update("jax_platforms", "cpu")
    import reference

    inputs = reference.setup_inputs()
    expected = np.asarray(reference.reference(**inputs))
    actual = kernel(**{k: np.asarray(v) for k, v in inputs.items()})
    err = actual.astype(np.float64) - expected.astype(np.float64)
    l2 = np.sqrt((err ** 2).mean()) / np.sqrt(
        (expected.astype(np.float64) ** 2).mean())
    print(f"Relative error: {l2:.6e}")
